# revision 85
# baseline (speedup 1.0000x reference)
"""Cross-attention (GQA) Trainium2 Bass kernel — pipelined, final.

Problem: B=2, Tq=Tkv=2048, D_MODEL=1024, 16 query heads / 4 kv heads,
head_dim=64.  Sharded over 8 NeuronCores as batch(2) x kv-group(4); each
core computes 4 query heads + its single kv head and a partial output
projection (Wo row-split by head group); partials are summed on host.

Dataflow (feature dim on SBUF partitions end-to-end, no big transposes):

  A: qT[e,t] = WqT.T @ xqT,  kvT = WkvT.T @ xcT   (weights stationary)
     v[tk,dv] via PE-transpose of vT tiles; vp=[v|1], vp2=[1|v]
  B: per (sec=blk,e) unit t: pb[128,1024] = two K=64 row-group matmuls
     (h_even rows 0-63 -> cols 0:512, h_odd rows 64-127 -> cols
     512:1024), concurrent in the PE array.
  C: pt = exp(pb/8) one ScalarE instruction per unit (FD=1024).
  D: pd_h[128,512] += vp_t.T @ pt_half; ones-columns give the softmax
     denominators in the complementary 64 partitions.
  E: yT += WoT.T @ (pd*recip(den)), row-split by head pair.

Structure (hardware-measured at ~190.4us, from a 226.7us baseline):
  - ONE continuous 128-unit software pipeline (B(u), D(u-lag), exp(u))
    across all 8 sections: no PE drain / exp restart bubble at section
    boundaries.  ScalarE exp (~143us busy) is the pacer.
  - reciprocal_approx_fast (custom DVE op, ~5x faster than RECIPROCAL)
    on a full-partition composed denominator tile (the op silently
    corrupts partition-offset operands), one recip + 2 swap DMAs + one
    full-width multiply per section.
  - All inputs host-pre-arranged contiguous-per-partition (~128 large
    descriptors per DMA, not 1024 small ones), issued unconditionally
    up front and split across BOTH hardware DGE queues (sync + Act);
    chunk 0 lands in quarters so the PE consumes data as it arrives.
    No waiting DMA ever sits in front of an input load: the Tile
    scheduler SINKS matmuls whose DMA deps it models as late (this
    single effect was worth ~17us).
  - k2 (kT copied into partitions 64-127 for the h64 row group) built
    with a block-swap identity matmul + aligned DVE copy instead of an
    SBUF->SBUF broadcast DMA, for the same scheduler reason.
  - Norm chains and output-projection (E) pieces pop from a dedicated
    "late" queue only at units t in [5..14] of the FOLLOWING section,
    when their DVE-side producers have long retired - they never stall
    the in-order PE.
  - y stores staged per tq-block in SBUF, 2 half-block DMAs per block
    (4 quarter stores for the last block, interleaved with the E
    matmuls); outputs in a partition-major layout un-permuted on host.
  - PE P-state management: the PE needs ~3us of CONTINUOUS execution to
    reach 2.4GHz (1.2GHz otherwise, 0.65 cold).  Warm-up matmuls cover
    preamble->first-data; tail warm-keepers run during the final norm
    chain so the last output projection executes at speed; the tail E
    uses a 4-bank PSUM rotation (psA pair + freed pd banks).
  - A DVE-offloaded exp path (2 chained custom DVE ops: poly3 of
    x/256, then (1+q)^32 by 5 squarings; ~7e-4 rel err) is implemented
    and hardware-validated but DISABLED: with pb capped at 2 PSUM bufs
    (matmul out must be fp32), B(t+2) gates on exp(t) completion and
    the in-order DVE queue delivers offloaded pt too late - measured
    net zero.  See off_t below.
"""

import os
import sys
from collections import deque

import numpy as np

for _p in ("/opt/trn_rl_repo",):
    if _p not in sys.path and os.path.isdir(_p):
        sys.path.insert(0, _p)

import concourse.bass as bass
import concourse.bacc as bacc
import concourse.mybir as mybir
from concourse.tile import TileContext

# ---------------------------------------------------------------- DVE exp ops
# Two custom DVE ops that together compute exp(x/8) = (1 + q)^32 with
# q = poly3(x/256) (Taylor-3 of e^{x/256}-1).  Offloading a few exp units
# per section to the DVE takes them off the ScalarE critical path (the
# ACT engine is the kernel's pacer).  Registered at import into the
# concourse dve_ops tables (name/opcode/spec), shas pinned from lower().
import concourse.dve_ops as _dve_ops
from concourse.dve_ops import DveOp as _DveOp
from concourse.dve_spec import (
    C0 as _C0, C1 as _C1, C2 as _C2, Spec as _Spec, Src0 as _Src0,
    _has_src1, lower as _lower, sq as _sq,
)
from concourse.dve_uop import DveOpSpec as _DveOpSpec

_S = 1.0 / 256.0
EXP_C0 = _S * _S * _S / 6.0
EXP_C1 = _S * _S / 2.0
EXP_C2 = _S


def _mk_dve_op(name, spec):
    if name in _dve_ops._SUB_OPCODE_FOR_NAME:
        return next(op for op in _dve_ops.OPS if op.name == name)
    opcode = _dve_ops._CUSTOM_DVE_ROW_BASE + len(_dve_ops.OPS)
    assert opcode < 0x20
    shas = {}
    for ver in ("v3", "v4"):
        try:
            s = _DveOpSpec(
                name=name, opcode=opcode, uops=_lower(spec, ver=ver),
                rd1_en=_has_src1(spec),
            )
            shas[ver] = s.sha(ver)
        except Exception:
            pass
    op = _DveOp(name, spec, False, shas)
    _dve_ops.OPS.append(op)
    _dve_ops.CUSTOM_DVE_SPECS[name] = spec
    _dve_ops._SUB_OPCODE_FOR_NAME[name] = opcode
    return op


EXP_POLY = _mk_dve_op(
    "EXP_POLY_ANT",
    _Spec(
        body=((_C0 * _Src0 + _C1) * _Src0 + _C2) * _Src0,
        reference=lambda in0, in1, c0, c1, c2: ((c0 * in0 + c1) * in0 + c2) * in0,
    ),
)
ONE_P_SQ5 = _mk_dve_op(
    "ONE_P_SQ5_ANT",
    _Spec(
        body=_sq(_sq(_sq(_sq(_sq(_Src0 + _C0))))),
        reference=lambda in0, in1, c0, c1, c2: (in0 + c0) ** 32,
    ),
)

# ---------------------------------------------------------------- problem dims
B = 2
TQ = 2048
TKV = 2048
D_MODEL = 1024
N_HEADS = 16
N_KV_HEADS = 4
HEAD_DIM = 64
N_CORES = 8
GROUPS = N_KV_HEADS  # kv groups = 4
HEADS_PER_DEV = N_HEADS // GROUPS  # 4
DQ = HEADS_PER_DEV * HEAD_DIM  # 256
DKV = 2 * HEAD_DIM  # 128 (k rows + v rows stacked)
SCALE = 1.0 / float(np.sqrt(HEAD_DIM))

P = 128
FREE = 512  # matmul moving-operand chunk / tq block width
BLK = 512
NBLK = TQ // BLK  # 4 tq blocks
DT = D_MODEL // P  # 8 d-tiles
ET = DQ // P  # 2 e-tiles (query head pairs)
NCH = TQ // FREE  # 4 x chunks of 512
NTK = TKV // P  # 16 tk tiles
MT = D_MODEL // P  # 8 output m-tiles
NSEC = NBLK * ET  # 8 sections
NU = NSEC * NTK  # 128 pipelined units

F32 = mybir.dt.float32
F16 = mybir.dt.float16


def build_bass():
    nc = bacc.Bacc()

    # all inputs/outputs are host-pre-arranged to be contiguous per SBUF
    # partition: each load/store is ~128 large descriptors, not 1024 small
    # ones (DGE issue cost and HBM efficiency both scale with that).
    xq = nc.declare_dram_parameter("xqh", [P, NCH, DT, FREE], F16, isOutput=False)
    xc = nc.declare_dram_parameter("xch", [P, NCH, DT, FREE], F16, isOutput=False)
    wq = nc.declare_dram_parameter("wqh", [P, ET, DT, P], F16, isOutput=False)
    wkv = nc.declare_dram_parameter("wkvh", [P, DT, DKV], F16, isOutput=False)
    wo = nc.declare_dram_parameter("woh", [P, ET, D_MODEL], F16, isOutput=False)
    # block-swap matrix [[0,I64],[I64,0]]: cid2[64:, :64] is a plain I64 for
    # PE transposes; the full matrix PE-shifts kT from partitions 0-63 into
    # 64-127 (replacing a scheduler-hostile SBUF->SBUF broadcast DMA).
    cid = nc.declare_dram_parameter("cid2", [P, P], F16, isOutput=False)
    yt = nc.declare_dram_parameter("yh", [P, NBLK, MT, FREE], F16, isOutput=True)
    # e0-half partial of the last block's output projection, computed during
    # section 7 (it only needs norm(6)) and summed into yh's block 3 on host
    yp = nc.declare_dram_parameter("yp", [P, MT, FREE], F16, isOutput=True)

    with TileContext(nc) as tc:
        with (
            tc.tile_pool(name="consts", bufs=1) as consts,
            tc.tile_pool(name="pt", bufs=10) as ptpool,
            tc.tile_pool(name="qx", bufs=2) as qxpool,
            tc.tile_pool(name="rec", bufs=2) as recpool,
            tc.tile_pool(name="yout", bufs=2) as ypool,
            tc.tile_pool(name="psS", bufs=2, space="PSUM") as psS,
            tc.tile_pool(name="psD", bufs=1, space="PSUM") as psD,
            tc.tile_pool(name="psA", bufs=2, space="PSUM") as psA,
        ):
            # ---------------- persistent tiles
            qt = consts.tile([P, ET, TQ], F16, tag="qt")  # head pair per e
            kv = consts.tile([P, TKV], F16, tag="kv")  # rows 0-63 kT, 64-127 vT
            k2 = consts.tile([P, TKV], F16, tag="k2")  # rows 64-127 = kT copy
            vp = consts.tile([P, NTK, P], F16, tag="vp")  # [v | ones]
            vp2 = consts.tile([P, NTK, P], F16, tag="vp2")  # [ones | v]
            outs = consts.tile([P, ET, TQ], F16, tag="outs")  # normalized outT
            ident = consts.tile([P, P], F16, tag="ident")
            wkv_sb = consts.tile([P, DT, DKV], F16, tag="wkv")
            wq_sb = consts.tile([P, ET, DT, P], F16, tag="wq")
            wo_sb = consts.tile([P, ET, D_MODEL], F16, tag="wo")
            wrm = consts.tile([P, P], F16, tag="wrm")
            xq_t = [
                consts.tile([P, DT, FREE], F16, tag=f"xq{c}", name=f"xq{c}")
                for c in range(NCH)
            ]
            xc_t = [
                consts.tile([P, DT, FREE], F16, tag=f"xc{c}", name=f"xc{c}")
                for c in range(NCH)
            ]

            # ---------------- input DMA burst: everything issues up front,
            # split across the two hardware DGE queues.  Neither queue has a
            # waiting DMA in front of an input load.  Most-urgent first.
            # sync queue: kv path (xc chunks feed the B-matmul deadline
            # chain), then the non-urgent q-path chunks.
            # chunk 0 lands in quarters (one per projection piece) so the PE
            # ramps continuously instead of stalling on whole-chunk arrivals
            nc.sync.dma_start(wkv_sb, wkv[:])
            for qi in range(4):
                nc.sync.dma_start(
                    xc_t[0][:, 2 * qi : 2 * qi + 2, :], xc[:, 0, 2 * qi : 2 * qi + 2, :]
                )
            H = DT // 2
            nc.sync.dma_start(xc_t[1][:, :H, :], xc[:, 1, :H, :])
            nc.sync.dma_start(xc_t[1][:, H:, :], xc[:, 1, H:, :])
            for c in range(2, NCH):
                nc.sync.dma_start(xc_t[c], xc[:, c, :, :])
            # wq's e1 half is only needed by the q(0,1) fills mid-section-0
            nc.sync.dma_start(wq_sb[:, 1, :, :], wq[:, 1, :, :])
            for c in range(1, NCH):
                nc.sync.dma_start(xq_t[c], xq[:, c, :, :])
            nc.sync.dma_start(wo_sb, wo[:])
            # scalar/Act queue (idle until the first exp): only what the
            # first B matmul needs, so the issue cost never delays exp(0).
            # ident first: it gates the k2 shift and the v transposes.
            nc.scalar.dma_start(ident, cid[:])
            nc.scalar.dma_start(wq_sb[:, 0, :, :], wq[:, 0, :, :])
            for qi in range(4):
                nc.scalar.dma_start(
                    xq_t[0][:, 2 * qi : 2 * qi + 2, :], xq[:, 0, 2 * qi : 2 * qi + 2, :]
                )

            # vector-side const init + HAM warm-up (no DMA dependencies:
            # wrm memset feeds dummy matmuls that spin the PE clock up while
            # the inputs stream; a tiny exp pulls the ACT table load early).
            nc.vector.memset(wrm, 0.125)
            nc.vector.memset(vp, 1.0)
            nc.vector.memset(vp2, 1.0)
            dum = consts.tile([P, 8], F16, tag="dum")
            nc.scalar.activation(
                dum, wrm[:, :8], mybir.ActivationFunctionType.Exp, bias=0.0, scale=1.0
            )
            warm = psA.tile([P, P], F32, tag="pa", name="warm")
            for i in range(26):
                nc.tensor.matmul(warm, wrm, wrm, start=(i == 0), stop=(i == 25))

            # ---------------- fill-work machinery (PE slack consumers)
            fills = deque()  # anytime work: projections, transposes, dma issues
            late = deque()  # dep-settled-late work: norm chains, E pieces
            tail_fills = deque()  # sec-7-only: E(3) e0-half partials

            def pop_fill(n=1):
                for _ in range(n):
                    if not fills:
                        return
                    fills.popleft()()

            # D matmuls for one pipelined unit (two heads, K=128, N=512)
            def emit_d(pd0, pd1, pt, t):
                nc.tensor.matmul(
                    pd0, vp[:, t, :], pt[:, :BLK],
                    start=(t == 0), stop=(t == NTK - 1), skip_group_check=True,
                )
                nc.tensor.matmul(
                    pd1, vp2[:, t, :], pt[:, BLK:],
                    start=(t == 0), stop=(t == NTK - 1), skip_group_check=True,
                )

            # kv projection chunk: 8 K-tiles -> kv[:, cs]; k2 shift; transposes
            # (chunk 0's PSUM->SBUF copy runs on the idle-during-lead-in
            # ScalarE so the DVE queue never delays the B(0) chain)
            def kv_chunk_pieces(c):
                cs = slice(c * FREE, (c + 1) * FREE)
                st = {}

                def pk(i0):
                    def p():
                        if i0 == 0:
                            st["pkv"] = psA.tile([P, FREE], F32, tag="pa", name="pkv")
                        for i in range(i0, i0 + 2):
                            nc.tensor.matmul(
                                st["pkv"], wkv_sb[:, i, :], xc_t[c][:, i, :],
                                start=(i == 0), stop=(i == DT - 1),
                            )
                        if i0 == DT - 2:
                            if c == 0:
                                nc.scalar.copy(kv[:, cs], st["pkv"])
                            else:
                                nc.vector.tensor_copy(kv[:, cs], st["pkv"])

                    return p

                def p4():
                    # kT -> partitions 64-127 of k2 via the block-swap matmul
                    psK = psA.tile([P, FREE], F32, tag="pa", name="psK")
                    nc.tensor.matmul(psK, ident[:HEAD_DIM, :], kv[:HEAD_DIM, cs])
                    nc.vector.tensor_copy(k2[HEAD_DIM:, cs], psK[HEAD_DIM:, :])

                def p3():
                    # transpose the 4 v tiles of this chunk, batch-copy to vp/vp2
                    pvb = psA.tile([P, 4 * HEAD_DIM], F16, tag="pa", name="pvb")
                    for k in range(4):
                        ts_ = slice((4 * c + k) * P, (4 * c + k + 1) * P)
                        nc.tensor.transpose(
                            pvb[:, k * HEAD_DIM : (k + 1) * HEAD_DIM],
                            kv[HEAD_DIM:, ts_],
                            ident[HEAD_DIM:, :HEAD_DIM],
                        )
                    src = pvb.rearrange("p (k d) -> p k d", k=4)
                    nc.vector.tensor_copy(vp[:, 4 * c : 4 * c + 4, :HEAD_DIM], src)
                    nc.vector.tensor_copy(vp2[:, 4 * c : 4 * c + 4, HEAD_DIM:], src)

                return [pk(0), pk(2), pk(4), pk(6), p4, p3]

            # q projection chunk (one e): 8 K-tiles -> qt[:, e, cs]
            def q_chunk_pieces(c, e, lead=False):
                cs = slice(c * FREE, (c + 1) * FREE)
                st = {}

                def pq(i0):
                    def p():
                        if i0 == 0:
                            st["pq"] = psA.tile([P, FREE], F32, tag="pa", name="pq")
                        for i in range(i0, i0 + 2):
                            nc.tensor.matmul(
                                st["pq"], wq_sb[:, e, i, :],
                                xq_t[c][:, i, :],
                                start=(i == 0), stop=(i == DT - 1),
                            )
                        if i0 == DT - 2:
                            if lead:
                                nc.scalar.copy(qt[:, e, cs], st["pq"])
                            else:
                                nc.vector.tensor_copy(qt[:, e, cs], st["pq"])

                    return p

                return [pq(0), pq(2), pq(4), pq(6)]

            # output projection for one tq block: 8 m-tile pieces staged into
            # ySB, then half-block DMA issues.  In tail mode the py PSUM
            # accumulators rotate over 4 banks (psA pair + the pd banks,
            # which are free once the final spill ran) so the E matmuls
            # never stall on the yo casts, and stores are finer-grained so
            # the last transfer is small.
            def e_pieces(blk, tail=False):
                bs = slice(blk * BLK, (blk + 1) * BLK)
                ysb = ypool.tile([P, MT, FREE], F16, tag="ysb", name=f"ysb{blk}")

                def mk(m):
                    def p():
                        if tail and m % 2 == 1:
                            tag = "pd0" if m % 4 == 1 else "pd1"
                            py = psD.tile([P, FREE], F32, tag=tag, name="py")
                        else:
                            py = psA.tile([P, FREE], F32, tag="pa", name="py")
                        for ee in range(ET):
                            nc.tensor.matmul(
                                py, wo_sb[:, ee, m * P : (m + 1) * P], outs[:, ee, bs],
                                start=(ee == 0), stop=(ee == ET - 1),
                            )
                        if tail:
                            # keeper pair: fills the cast-wait bubbles so the
                            # PE P-state holds full clock through the chain
                            kw = psS.tile([P, 2 * BLK], F32, tag="pb", name="kw")
                            nc.tensor.matmul(kw[:, :P], outs[:, 0, bs][:, :P],
                                             outs[:, 1, bs][:, :P])
                        nc.vector.tensor_copy(ysb[:, m, :], py)

                    return p

                def store(h, nst):
                    def p():
                        ms = slice(h * (MT // nst), (h + 1) * (MT // nst))
                        nc.sync.dma_start(yt[:, blk, ms, :], ysb[:, ms, :])

                    return p

                def wave(ms):
                    # all e0 matmuls of the wave first: they only need
                    # outs[:,0] (ready since norm(6)) and run while the
                    # final norm muls still occupy the DVE; the e1 halves
                    # follow once outs[:,1] lands.
                    pys = {}

                    def p():
                        for m in ms:
                            if m % 2 == 1:
                                tag = "pd0" if m % 4 == 1 else "pd1"
                                pys[m] = psD.tile([P, FREE], F32, tag=tag,
                                                  name="py")
                            else:
                                pys[m] = psA.tile([P, FREE], F32, tag="pa",
                                                  name="py")
                            nc.tensor.matmul(
                                pys[m], wo_sb[:, 0, m * P : (m + 1) * P],
                                outs[:, 0, bs], start=True, stop=False,
                            )
                        for m in ms:
                            nc.tensor.matmul(
                                pys[m], wo_sb[:, 1, m * P : (m + 1) * P],
                                outs[:, 1, bs], start=False, stop=True,
                            )
                            kw = psS.tile([P, 2 * BLK], F32, tag="pb",
                                          name="kw")
                            nc.tensor.matmul(kw[:, :P], outs[:, 0, bs][:, :P],
                                             outs[:, 1, bs][:, :P])
                            nc.vector.tensor_copy(ysb[:, m, :], pys[m])

                    return p

                if tail:
                    return [
                        wave(range(0, 4)), store(0, 4), store(1, 4),
                        wave(range(4, 8)), store(2, 4),
                        store(MT - 2, MT), store(MT - 1, MT),
                    ]
                return [mk(m) for m in range(MT)] + [store(0, 2), store(1, 2)]

            # E(3) e0-half partial: one start&stop matmul + cast per m-tile,
            # staged into ysb0 and stored to the yp partial output.  Runs as
            # section-7 fills (popped only after norm(6)'s emission).
            ysb0 = consts.tile([P, MT, FREE], F16, tag="ysb0")

            def e0_pieces():
                bs = slice((NBLK - 1) * BLK, NBLK * BLK)

                def mk0(m):
                    def p():
                        py0 = psA.tile([P, FREE], F32, tag="pa", name="py0")
                        nc.tensor.matmul(py0, wo_sb[:, 0, m * P : (m + 1) * P],
                                         outs[:, 0, bs])
                        nc.vector.tensor_copy(ysb0[:, m, :], py0)

                    return p

                def st(h):
                    def p():
                        ms = slice(h * (MT // 2), (h + 1) * (MT // 2))
                        nc.sync.dma_start(yp[:, ms, :], ysb0[:, ms, :])

                    return p

                out = [mk0(m) for m in range(MT)]
                out.insert(4, st(0))
                out.append(st(1))
                return out

            # spill pd0/pd1 for one section into aligned full-partition
            # tiles: rawN = [AV_even | AV_odd], rawD = [den_odd | den_even]
            # (reciprocal_approx_fast silently corrupts partition-offset
            # operands, so the custom op must see full offset-0 tiles).
            def spill(dp0, dp1):
                rawN = recpool.tile([P, BLK], F32, tag="rawN", name="rawN")
                rawD = recpool.tile([P, BLK], F32, tag="rawD", name="rawD")
                nc.vector.tensor_copy(rawN[:HEAD_DIM, :], dp0[:HEAD_DIM, :])
                nc.vector.tensor_copy(rawN[HEAD_DIM:, :], dp1[HEAD_DIM:, :])
                nc.vector.tensor_copy(rawD[:HEAD_DIM, :], dp1[:HEAD_DIM, :])
                nc.vector.tensor_copy(rawD[HEAD_DIM:, :], dp0[HEAD_DIM:, :])
                return rawN, rawD

            # normalize chain for one section (deferred into the late window
            # of the following section): one full-partition fast recip, two
            # half-swap broadcast DMAs, one full-width multiply.
            def norm_chain(sec, rawN, rawD):
                blk, e = divmod(sec, ET)
                bs = slice(blk * BLK, (blk + 1) * BLK)

                def p():
                    recD = recpool.tile([P, BLK], F32, tag="recD", name="recD")
                    recS = recpool.tile([P, BLK], F32, tag="recS", name="recS")
                    nc.vector.reciprocal_approx_fast(recD, rawD)
                    nc.sync.dma_start(recS[:HEAD_DIM, :], recD[HEAD_DIM:, :])
                    nc.sync.dma_start(recS[HEAD_DIM:, :], recD[:HEAD_DIM, :])
                    nc.vector.tensor_mul(outs[:, e, bs], rawN, recS)

                p.is_norm = True
                return p

            # ---------------- lead-in PE work: kv chunk 0 + q chunk 0 (e=0)
            # inline; everything else is fills with deadline-ordered layout.
            # interleaved so the PE consumes each quarter-chunk DMA as it
            # lands; kc0 finishes first (its post-matmul chain kv-copy ->
            # k2-shift -> transposes is longer than q00's single cast).
            kc0 = kv_chunk_pieces(0)
            q00 = q_chunk_pieces(0, 0, lead=True)
            kc0[0]()
            kc0[1]()
            q00[0]()
            kc0[2]()
            q00[1]()
            kc0[3]()
            kc0[4]()  # k2 shift for chunk 0 (B(0) reads it)
            q00[2]()
            q00[3]()

            # Fill deadline order for section 0 (2 pops/unit):
            #   kc0.p3 (v transposes, before D(0) at u=2) -> t=0
            #   kc{c}: pk x4 + p4 (k2 shift) before B(4c); p3 before D(4c).
            # deadline-exact order: each kv chunk's pk/p4 land just before
            # their consuming B(4c); the v-transpose p3 pieces slot into the
            # relaxed D deadlines (D-lag 8 in section 0); q projections
            # last.  SEC0_POPS below releases exactly this schedule so the
            # lead-in PE queue ahead of each stalling B stays minimal.
            kc1 = kv_chunk_pieces(1)
            kc2 = kv_chunk_pieces(2)
            kc3 = kv_chunk_pieces(3)
            fills.extend(kc1[:5])   # by B(4)
            fills.append(kc0[5])    # v transposes chunk 0, by D(0) @ u8
            fills.extend(kc2[:5])   # by B(8)
            fills.append(kc1[5])    # by D(4) @ u12
            fills.extend(kc3[:5])   # by B(12)
            fills.append(kc2[5])    # by D(8) @ u16
            fills.extend(q_chunk_pieces(0, 1))
            fills.append(kc3[5])    # by D(12) @ u20 (drains early sec 1)
            for e in range(ET):
                fills.extend(q_chunk_pieces(1, e))
            for e in range(ET):
                fills.extend(q_chunk_pieces(2, e))
            for e in range(ET):
                fills.extend(q_chunk_pieces(3, e))
            SEC0_POPS = (2, 2, 1, 1, 2, 2, 1, 1, 2, 2, 1, 1, 1, 2, 2, 1)
            # e0_pieces (tail e0-half overlap into section 7) measured
            # slower on hardware — sec-7 DVE/PSUM contention ate the tail
            # saving.  Left implemented but not scheduled; yp is unused.

            # ---------------- the continuous BCD pipeline over 128 units
            units = [(sec, t) for sec in range(NSEC) for t in range(NTK)]
            pending = deque()  # (pd0, pd1, pt, t, sec, u, offloaded)
            pd_cur = None
            norm_at5 = False

            def drain_ready(u):
                # D-lag is 2 for ScalarE exp units but 6 for DVE-offloaded
                # ones (the in-order DVE queue delivers their pt later).
                # Section 0 runs at lag 8: its Ds aren't on the exp critical
                # path until the first spill, and deferring them gives the
                # oversubscribed lead-in PE room for the kv deadline fills.
                while pending:
                    _, _, _, _, _, pu, off = pending[0]
                    # (lag 8 extended through section 1 was measured: it
                    # halves section 1's stall but the transition burst
                    # resurfaces larger in section 2 — net slower.)
                    lag = 8 if pu < NTK else (6 if off else 2)
                    if u - pu < lag:
                        return
                    dp0, dp1, dpt, dt_, dsec, _, _ = pending.popleft()
                    emit_d(dp0, dp1, dpt, dt_)
                    if dt_ == NTK - 1 and dsec < NSEC - 1:
                        # section dsec fully accumulated: spill pd -> raw
                        # (frees the PSUM banks for this section's own Ds),
                        # queue the normalize + block-complete E work.
                        rawN, rawD = spill(dp0, dp1)
                        # norm jumps the queue: its deps settle first, and
                        # section 7's E(3)-e0 partials need norm(6) emitted
                        # by (7,5) (leftover E pieces would otherwise hold
                        # it back to (7,15)).
                        late.appendleft(norm_chain(dsec, rawN, rawD))
                        if dsec % ET == ET - 1 and dsec >= 1:
                            late.extend(e_pieces(dsec // ET))

            for u, (sec, t) in enumerate(units):
                blk, e = divmod(sec, ET)
                bs = slice(blk * BLK, (blk + 1) * BLK)
                # DVE-offloaded exp units (custom ops above).  Measured on
                # hardware: with pb limited to 2 PSUM bufs, B(t+2) gates on
                # exp(t) completion, and the in-order DVE queue delivers the
                # offloaded pt too late — every placement tried traded the
                # saved ScalarE time for equal PE/exp stalls.  Disabled.
                off_t = ()
                if t == 0:
                    pd_cur = (
                        psD.tile([P, BLK], F32, tag="pd0", name="pd0"),
                        psD.tile([P, BLK], F32, tag="pd1", name="pd1"),
                    )
                pb = psS.tile([P, 2 * BLK], F32, tag="pb", name="pb")
                # B: two K=64 row-group matmuls, concurrent in the array
                nc.tensor.matmul(pb[:, :BLK], kv[:HEAD_DIM, t * P : (t + 1) * P],
                                 qt[:HEAD_DIM, e, bs])
                nc.tensor.matmul(pb[:, BLK:], k2[HEAD_DIM:, t * P : (t + 1) * P],
                                 qt[HEAD_DIM:, e, bs])
                drain_ready(u)
                pt = ptpool.tile([P, 2 * BLK], F16, tag="pt", name="pt")
                offloaded = t in off_t
                if offloaded:
                    qx = qxpool.tile([P, 2 * BLK], F32, tag="qx", name="qx")
                    nc.vector._custom_dve(
                        EXP_POLY, out=qx, in0=pb, s0=EXP_C0, s1=EXP_C1, imm2=EXP_C2
                    )
                    nc.vector._custom_dve(ONE_P_SQ5, out=pt, in0=qx, s0=1.0)
                else:
                    nc.scalar.activation(
                        pt, pb, mybir.ActivationFunctionType.Exp, bias=0.0, scale=SCALE
                    )
                pending.append((pd_cur[0], pd_cur[1], pt, t, sec, u, offloaded))
                last_pt = pt
                if sec == 0:
                    pop_fill(SEC0_POPS[t])
                elif t >= 5 and t % 2 == 1 and late:
                    # every OTHER unit: the E-piece psA rotation needs its
                    # yo cast (DVE) done before the matmul two pieces later;
                    # 1-unit spacing let the norm chain back the casts up
                    # and stall the in-order PE.  (Skipping t=7 after a
                    # t=5 norm pop + using idle even slots for late work
                    # was measured net slower — the displaced E pieces
                    # stall elsewhere.)
                    late.popleft()()
                elif t >= 2 and t % 2 == 0:
                    # even units only: section 1 is PE-capacity-tight (it
                    # absorbs section 0's deferred Ds); the q projections'
                    # deadlines are a section+ away, so let them spill.
                    pop_fill(1)

            # ---------------- tail: drain last two Ds, normalize the final
            # section straight out of PSUM (fast recip), output-project the
            # last block, store.
            final_pd = None
            while pending:
                dp0, dp1, dpt, dt_, dsec, _, _ = pending.popleft()
                emit_d(dp0, dp1, dpt, dt_)
                final_pd = (dp0, dp1)
            # leftover e0 partials drain first (useful PE warm-keeping: they
            # only need outs[:,0], long ready), then dep-pinned keepers
            # cover the rest of the norm-chain window.
            while tail_fills:
                tail_fills.popleft()()
            wt = psA.tile([P, P], F32, tag="pa", name="wt")
            for i in range(20):
                nc.tensor.matmul(
                    wt, last_pt[:, :P], last_pt[:, :P],
                    start=(i == 0), stop=(i == 19),
                )
            while late:
                late.popleft()()
            # tail normalize, minimum latency: den spill only, fast recip,
            # fp16 cast + PE block-swap (no DMA round trip), muls straight
            # from the pd PSUM banks.
            dp0, dp1 = final_pd
            e, bs = 1, slice((NBLK - 1) * BLK, NBLK * BLK)
            rawD = recpool.tile([P, BLK], F32, tag="rawD", name="rawD")
            nc.vector.tensor_copy(rawD[:HEAD_DIM, :], dp1[:HEAD_DIM, :])
            nc.vector.tensor_copy(rawD[HEAD_DIM:, :], dp0[HEAD_DIM:, :])
            recD = recpool.tile([P, BLK], F32, tag="recD", name="recD")
            nc.vector.reciprocal_approx_fast(recD, rawD)
            recH = recpool.tile([P, BLK], F16, tag="recS", name="recH")
            nc.vector.tensor_copy(recH, recD)
            psR = psA.tile([P, BLK], F32, tag="pa", name="psR")
            nc.tensor.matmul(psR, ident, recH)
            recS = recpool.tile([P, BLK], F32, tag="rawN", name="recSf")
            nc.vector.tensor_copy(recS, psR)
            nc.vector.tensor_mul(outs[:HEAD_DIM, e, bs], dp0[:HEAD_DIM, :],
                                 recS[:HEAD_DIM, :])
            nc.vector.tensor_mul(outs[HEAD_DIM:, e, bs], dp1[HEAD_DIM:, :],
                                 recS[HEAD_DIM:, :])
            for piece in e_pieces(NBLK - 1, tail=True):
                piece()
            while fills:
                pop_fill()

    nc.finalize()
    return nc


_NC_CACHE = None


def _get_nc():
    global _NC_CACHE
    if _NC_CACHE is None:
        _NC_CACHE = build_bass()
    return _NC_CACHE


def _cid2():
    z = np.zeros((HEAD_DIM, HEAD_DIM), dtype=np.float16)
    i = np.eye(HEAD_DIM, dtype=np.float16)
    return np.block([[z, i], [i, z]])


def _chunked(xT):
    """[D_MODEL, T] -> [P, NCH, DT, FREE] with row i*P+p at [p, :, i, :]:
    each partition's chunk data contiguous for large-descriptor DMA."""
    return np.ascontiguousarray(
        xT.reshape(DT, P, NCH, FREE).transpose(1, 2, 0, 3)
    ).astype(np.float16)


def _wtiles(wT):
    """[D_MODEL, E] -> [P, DT, E]"""
    return np.ascontiguousarray(
        wT.reshape(DT, P, wT.shape[1]).transpose(1, 0, 2)
    ).astype(np.float16)


def shard_inputs(query, context, Wq, Wk, Wv, Wo):
    """host-side sharding: 8 cores = batch(2) x kv-group(4)"""
    in_maps = []
    xqh = [_chunked(np.asarray(query[b]).T) for b in range(B)]
    xch = [_chunked(np.asarray(context[b]).T) for b in range(B)]
    for core in range(N_CORES):
        b, g = divmod(core, GROUPS)
        wqT = Wq[g * DQ : (g + 1) * DQ, :].T  # [D_MODEL, DQ]
        wqh = np.ascontiguousarray(
            wqT.reshape(DT, P, ET, P).transpose(1, 2, 0, 3)
        ).astype(np.float16)
        wkvh = _wtiles(
            np.concatenate(
                [
                    Wk[g * HEAD_DIM : (g + 1) * HEAD_DIM, :],
                    Wv[g * HEAD_DIM : (g + 1) * HEAD_DIM, :],
                ],
                axis=0,
            ).T
        )
        woT = Wo[:, g * DQ : (g + 1) * DQ].T  # [DQ, D_MODEL]
        woh = np.ascontiguousarray(
            woT.reshape(ET, P, D_MODEL).transpose(1, 0, 2)
        ).astype(np.float16)
        in_maps.append(
            {
                "xqh": xqh[b],
                "xch": xch[b],
                "wqh": wqh,
                "wkvh": wkvh,
                "woh": woh,
                "cid2": _cid2(),
            }
        )
    return in_maps


def kernel(query, context, Wq, Wk, Wv, Wo, _want_profile=False):
    from concourse.bass_utils import run_bass_kernel_spmd

    nc = _get_nc()
    in_maps = shard_inputs(query, context, Wq, Wk, Wv, Wo)
    res = run_bass_kernel_spmd(
        nc, in_maps, core_ids=list(range(N_CORES)), trace=_want_profile
    )
    out = np.zeros((B, TQ, D_MODEL), dtype=np.float32)
    for core in range(N_CORES):
        b = core // GROUPS
        yh = res.results[core]["yh"].astype(np.float32)
        yT = yh.transpose(2, 0, 1, 3).reshape(D_MODEL, TQ)
        out[b] += yT.T
    if _want_profile:
        return out, res
    return out
